# revision 1
# baseline (speedup 1.0000x reference)
"""CTRNN forward kernel for 8 Trainium2 NeuronCores.

Strategy (time-parallel): the T=2000 scan is split into 8 segments of 256
steps (one per core, 8*256=2048 >= 2000; the tail overhang is padded with
zeros and discarded). Each core first runs 192 warmup steps from h=0 to
converge onto the true trajectory (the CTRNN is strongly contracting:
Jacobian norm ~0.93/step, so the warmup error is ~1e-6), then its 256
real steps.

Device formulation (scaled state): with g_s = 0.9^(-s) h_s the update
h_{s+1} = 0.9 h_s + 0.1(inp_s + W_hh relu(h_s) + b) becomes a pure
accumulation  g_{s+1} = g_s + W_in'' x~_s + W_hh'' relu(g_s)
(relu is positively homogeneous, so the 0.9^(-s) scale folds into the
weights and the host-prescaled input columns x~). g accumulates IN PLACE
in PSUM across all 448 steps via start=False matmuls; per step the only
non-matmul work is relu(g) (PSUM->SBUF, split ACT/DVE) and, for the 256
output steps, a copy of g to SBUF history. The output projection
W_out @ g_hist runs as column-packed matmuls every 8 steps; the host
applies the 0.9^(s+1) unscaling and adds b_out.
"""

import os
import sys
import types

import numpy as np

INPUT_SIZE = 64
HIDDEN = 256
OUT = 32
NUM_TASKS = 8
ALPHA = 0.1
DECAY = 1.0 - ALPHA

B = 256
T = 2000
N_CORES = 8
SEG = 256  # segment steps per core
WARM = 128  # warmup steps
STEPS = SEG + WARM  # 384
CH = 8  # steps per output chunk (one psum y bank)
NCH_SEG = SEG // CH  # 32 output chunks
DMA_STEPS = 48  # xt DMA chunk (384 = 8*48)
EPOCH = 64  # psum rescale period (bounds the 0.9^-s scaling for fp16)
D_AUG = INPUT_SIZE + NUM_TASKS + 1  # 73 (ones row carries the bias)
YG = 8  # y chunks per y DMA


def _install_ntff_hook():
    """Recreate the missing antenv.axon_hooks so trace=True can profile."""
    if "antenv.axon_hooks" in sys.modules:
        return
    mod = types.ModuleType("antenv.axon_hooks")
    mod._hook = None
    mod.set_axon_ntff_profile_hook = lambda h: setattr(mod, "_hook", h)
    mod.get_axon_ntff_profile_hook = lambda: mod._hook
    sys.modules["antenv.axon_hooks"] = mod
    try:
        from trn_agent_boot.trn_boot import _ntff_profile_via_ctypes

        mod.set_axon_ntff_profile_hook(
            _ntff_profile_via_ctypes("/opt/axon/libaxon_pjrt.so")
        )
    except Exception:
        pass


_install_ntff_hook()

import concourse.bacc as bacc
import concourse.tile as tile
import concourse.mybir as mybir
from concourse.bass_utils import run_bass_kernel_spmd

F32 = mybir.dt.float32
F32R = mybir.dt.float32r
BF16 = mybir.dt.bfloat16
F16 = mybir.dt.float16

LAST_RESULT = None  # test.py reads exec_time_ns from here

_PROGRAM = None


def build_program():
    from contextlib import ExitStack

    nc = bacc.Bacc("TRN2", target_bir_lowering=False, debug=False)

    xt_d = nc.dram_tensor("xt", [D_AUG, STEPS * B], F16, kind="ExternalInput")
    wi_d = nc.dram_tensor("wi", [D_AUG, 2 * 128], F16, kind="ExternalInput")
    wh_d = nc.dram_tensor("wh", [128, 2 * 2 * 128], F16, kind="ExternalInput")
    whe_d = nc.dram_tensor("whe", [128, 2 * 2 * 128], F16, kind="ExternalInput")
    wo_d = nc.dram_tensor("wo", [128, 2 * OUT], F16, kind="ExternalInput")
    y_d = nc.dram_tensor("y", [128, NCH_SEG * 2 * B], F32, kind="ExternalOutput")

    with tile.TileContext(nc) as tc:
        ctx = ExitStack()
        with ctx:
            const = ctx.enter_context(tc.tile_pool(name="const", bufs=1))
            xpool = ctx.enter_context(tc.tile_pool(name="xin", bufs=2))
            ppool = ctx.enter_context(tc.tile_pool(name="P", bufs=1, space="PSUM"))
            ypp = ctx.enter_context(tc.tile_pool(name="ypsum", bufs=2, space="PSUM"))
            rpool = ctx.enter_context(tc.tile_pool(name="r", bufs=3))
            gpool = ctx.enter_context(tc.tile_pool(name="ghist", bufs=2))
            ysb = ctx.enter_context(tc.tile_pool(name="ysb", bufs=2))

            wi = const.tile([D_AUG, 2, 128], F16)
            nc.sync.dma_start(wi[:], wi_d.ap().rearrange("p (a m) -> p a m", a=2))
            wh = const.tile([128, 2, 2, 128], F16)
            nc.sync.dma_start(
                wh[:], wh_d.ap().rearrange("p (a b m) -> p a b m", a=2, b=2)
            )
            whe = const.tile([128, 2, 2, 128], F16)
            nc.sync.dma_start(
                whe[:], whe_d.ap().rearrange("p (a b m) -> p a b m", a=2, b=2)
            )
            wo = const.tile([128, 2, OUT], F16)
            nc.sync.dma_start(wo[:], wo_d.ap().rearrange("p (a m) -> p a m", a=2))

            P = [
                ppool.tile(
                    [128, B],
                    F32,
                    name=f"P{jb}",
                    tag=f"P{jb}",
                    padded_shape=[128, 2 * B],  # full psum bank: no bank sharing
                )
                for jb in range(2)
            ]

            xt_r = xt_d.ap().rearrange("p (c n) -> p c n", n=DMA_STEPS * B)

            r_prev = None
            g_hist = None
            g_prev = None
            yp_cur = None
            y_sbuf = None
            x_sbuf = None

            for s in range(STEPS):
                c, sl = divmod(s, CH)  # output-chunk index
                dc, ds = divmod(s, DMA_STEPS)  # x-DMA chunk index / step within

                if ds == 0:
                    x_sbuf = xpool.tile([D_AUG, DMA_STEPS * B], F16, tag="x")
                    nc.sync.dma_start(x_sbuf[:], xt_r[:, dc, :])
                xs = x_sbuf[:, ds * B : (ds + 1) * B]

                if s > 0 and s % EPOCH == 0:
                    resc = float(DECAY**EPOCH)
                    for jb in range(2):
                        nc.vector.tensor_scalar_mul(P[jb][:], P[jb][:], resc)

                # ---- accumulate this step's contributions into P ----
                # bank-0 writers first so relu0(s+1) can start while bank-1
                # writers stream; kb1 matmuls wait on relu1 which runs on DVE
                # in parallel with ACT's relu0.
                whx = whe if (s > 0 and s % EPOCH == 0) else wh
                for jb in range(2):
                    nc.tensor.matmul(
                        P[jb][:],
                        wi[:, jb, :],
                        xs,
                        start=(s == 0),
                        stop=False,
                        skip_group_check=True,
                    )
                    if s > 0:
                        for kb in range(2):
                            nc.tensor.matmul(
                                P[jb][:],
                                whx[:, kb, jb, :],
                                r_prev[kb][:],
                                start=False,
                                stop=False,
                                skip_group_check=True,
                            )

                # ---- read P: relu (ACT||DVE) + g_hist copy (DVE/ACT) ----
                r0 = rpool.tile([128, B], F16, tag="r0")
                r1 = rpool.tile([128, B], F16, tag="r1")
                nc.scalar.activation(
                    r0[:], P[0][:], mybir.ActivationFunctionType.Relu
                )
                if s >= WARM:
                    cs, csl = divmod(s - WARM, CH)
                    if csl == 0:
                        g_hist = gpool.tile([128, 2, CH, B], F16, tag="g")
                    hsc = float(DECAY ** (s % EPOCH + 1))
                    # DVE: hist0 (ready at bank0-done) then relu1 (bank1-done);
                    # ACT: relu0 then hist1 — keeps both engines stall-free and
                    # releases each bank's WAR as early as possible.
                    nc.vector.tensor_scalar_mul(g_hist[:, 0, csl, :], P[0][:], hsc)
                    nc.vector.tensor_scalar_max(r1[:], P[1][:], 0.0)
                    nc.scalar.activation(
                        g_hist[:, 1, csl, :],
                        P[1][:],
                        mybir.ActivationFunctionType.Copy,
                        scale=hsc,
                    )
                else:
                    nc.vector.tensor_scalar_max(r1[:], P[1][:], 0.0)
                r_prev = (r0, r1)

                # ---- y GEMM: one matmul per step against the previous
                # chunk's g_hist (fills the PE gap in the chain tail) ----
                if s >= WARM + CH:
                    m = s % CH
                    cs = (s - WARM) // CH - 1  # chunk being projected
                    kb, t4 = divmod(m, 4)
                    if m == 0:
                        yp = ypp.tile([128, 2 * B], F32, tag="yp")
                        yp_cur = yp
                    nc.tensor.matmul(
                        yp_cur[32 * t4 : 32 * (t4 + 1), :],
                        wo[:, kb, :],
                        g_prev[:, kb, 2 * t4 : 2 * t4 + 2, :],
                        start=(kb == 0),
                        stop=(kb == 1),
                        skip_group_check=True,
                        tile_position=(0, 32 * t4),
                    )
                    if m == CH - 1:
                        if cs % YG == 0:
                            y_sbuf = ysb.tile([128, YG * 2 * B], F32, tag="y")
                        ysl = y_sbuf[:, (cs % YG) * 2 * B : (cs % YG + 1) * 2 * B]
                        if cs % 2 == 0:
                            nc.scalar.activation(
                                ysl, yp_cur[:], mybir.ActivationFunctionType.Copy
                            )
                        else:
                            nc.vector.tensor_copy(ysl, yp_cur[:])
                        if cs % YG == YG - 1:
                            g0 = (cs // YG) * YG * 2 * B
                            nc.sync.dma_start(
                                y_d.ap()[:, g0 : g0 + YG * 2 * B], y_sbuf[:]
                            )
                if s >= WARM and (s - WARM) % CH == CH - 1:
                    g_prev = g_hist

            # ---- tail: project the final chunk ----
            yp = ypp.tile([128, 2 * B], F32, tag="yp")
            for kb in range(2):
                for t4 in range(4):
                    nc.tensor.matmul(
                        yp[32 * t4 : 32 * (t4 + 1), :],
                        wo[:, kb, :],
                        g_prev[:, kb, 2 * t4 : 2 * t4 + 2, :],
                        start=(kb == 0),
                        stop=(kb == 1),
                        skip_group_check=True,
                        tile_position=(0, 32 * t4),
                    )
            cs = NCH_SEG - 1
            ysl = y_sbuf[:, (cs % YG) * 2 * B : (cs % YG + 1) * 2 * B]
            nc.scalar.activation(ysl, yp[:], mybir.ActivationFunctionType.Copy)
            g0 = (cs // YG) * YG * 2 * B
            nc.sync.dma_start(y_d.ap()[:, g0 : g0 + YG * 2 * B], y_sbuf[:])
    nc.finalize()
    return nc


def _get_program():
    global _PROGRAM
    if _PROGRAM is None:
        _PROGRAM = build_program()
    return _PROGRAM


def kernel(x, task_id, W_in, b_in, W_hh, b_hh, W_out, b_out):
    x = np.asarray(x, np.float32)
    task_id = np.asarray(task_id, np.float32)
    W_in = np.asarray(W_in, np.float32)
    b_in = np.asarray(b_in, np.float32)
    W_hh = np.asarray(W_hh, np.float32)
    b_hh = np.asarray(b_hh, np.float32)
    W_out = np.asarray(W_out, np.float32)
    b_out = np.asarray(b_out, np.float32)

    import ml_dtypes

    # ---- weights (shared across cores) ----
    # wi: lhsT [73, 256] = 0.1 * [W_in | b_in+b_hh]^T
    wi = np.zeros((D_AUG, HIDDEN), np.float32)
    wi[: INPUT_SIZE + NUM_TASKS, :] = ALPHA * W_in.T
    wi[INPUT_SIZE + NUM_TASKS, :] = ALPHA * (b_in + b_hh)
    # wh: lhsT [k, (kb, jb, j)] = (0.1/0.9) * W_hh[jb*128+j, kb*128+k]
    whs = (ALPHA / DECAY) * W_hh  # [j_out, k_in]
    wh = np.empty((128, 2, 2, 128), np.float32)
    for kb in range(2):
        for jb in range(2):
            wh[:, kb, jb, :] = whs[jb * 128 : (jb + 1) * 128, kb * 128 : (kb + 1) * 128].T
    wh_in = np.ascontiguousarray(wh.reshape(128, 512)).astype(np.float16)
    # at epoch-boundary steps the relu rhs was produced before the 0.9^EPOCH
    # rescale of P, so those steps use weights pre-scaled by 0.9^EPOCH
    whe_in = np.ascontiguousarray(wh.reshape(128, 512) * (DECAY**EPOCH)).astype(np.float16)
    # wo: lhsT [k, (kb, o)] = W_out[o, kb*128+k]
    wo = np.empty((128, 2, OUT), np.float32)
    for kb in range(2):
        wo[:, kb, :] = W_out[:, kb * 128 : (kb + 1) * 128].T
    wo_in = np.ascontiguousarray(wo.reshape(128, 2 * OUT)).astype(np.float16)

    # ---- per-core scaled input blocks ----
    # combined_aug[d, t, b]: [73, T, B]
    comb = np.concatenate(
        [x, np.broadcast_to(task_id[:, None, :], (B, T, NUM_TASKS))], axis=2
    )  # [B, T, 72]
    comb_t = comb.transpose(2, 1, 0)  # [72, T, B]
    # per-step scale 0.9^-(s+1) with s local to each core
    sc = (DECAY ** -(np.arange(STEPS, dtype=np.float64) % EPOCH + 1)).astype(np.float32)

    in_maps = []
    for core in range(N_CORES):
        seg0 = core * SEG  # global start of this core's segment
        t0 = seg0 - WARM
        xt = np.zeros((D_AUG, STEPS, B), np.float32)
        lo = max(t0, 0)
        hi = min(seg0 + SEG, T)
        if hi > lo:
            ls, le = lo - t0, hi - t0
            xt[: INPUT_SIZE + NUM_TASKS, ls:le, :] = comb_t[:, lo:hi, :]
            xt[INPUT_SIZE + NUM_TASKS, ls:le, :] = 1.0
        xt *= sc[None, :, None]
        in_maps.append(
            {
                "xt": np.ascontiguousarray(xt.reshape(D_AUG, STEPS * B)).astype(np.float16),
                "wi": np.ascontiguousarray(wi).astype(np.float16),
                "wh": wh_in,
                "whe": whe_in,
                "wo": wo_in,
            }
        )

    nc = _get_program()
    global LAST_RESULT
    trace = bool(int(os.environ.get("KERNEL_TRACE", "0")))
    LAST_RESULT = run_bass_kernel_spmd(
        nc, in_maps, core_ids=list(range(N_CORES)), trace=trace
    )

    # ---- gather: y[(t4,o), (chunk, s2, b)] -> [B, T, OUT] ----
    out = np.empty((B, T, OUT), np.float32)
    # device already applied the 0.9^(s+1) unscale in the hist extract
    for core in range(N_CORES):
        y = LAST_RESULT.results[core]["y"].astype(np.float64)
        y = y.reshape(4, OUT, NCH_SEG, 2, B)  # [(t4, o), (chunk, s2, b)]
        y = y.transpose(2, 0, 3, 1, 4).reshape(SEG, OUT, B)  # [i, o, b]
        y = y + b_out.astype(np.float64)[None, :, None]
        seg0 = core * SEG
        n = min(SEG, T - seg0)
        if n > 0:
            out[:, seg0 : seg0 + n, :] = y[:n].transpose(2, 0, 1).astype(np.float32)
    return out



# revision 3
# speedup vs baseline: 1.3116x; 1.3116x over previous
"""CTRNN forward kernel for 8 Trainium2 NeuronCores.

Strategy (time-parallel, 2 staggered chains per core): the T=2000 scan is
split into 16 segments of 128 steps (2 per core; 16*128=2048 >= 2000, the
tail overhang is zero-padded and discarded). Each segment first runs W
warmup steps from h=0 to converge onto the true trajectory (the CTRNN
contracts at ~0.928/step), then its 128 real steps.

Device formulation (scaled state): with g_s = 0.9^(-s) h_s the update
h_{s+1} = 0.9 h_s + 0.1(inp_s + W_hh relu(h_s) + b) becomes a pure
accumulation  g_{s+1} = g_s + W_in'' x~_s + W_hh'' relu(g_s)
(relu is positively homogeneous, so the 0.9^(-s) scale folds into the
weights and the host-prescaled input columns x~). Each chain's g
accumulates IN PLACE in one PSUM bank across all W+128 steps via
start=False matmuls.

The two chains run the same local step half-a-round apart: while chain A's
PSUM bank is being read (one merged ACT relu over both hidden halves + one
merged DVE scaled-copy into the g history), chain B's six 256-wide f16
matmuls keep the PE busy, so the per-step relu->matmul latency that
dominated the single-chain version is hidden and the PE stays at full
clock. The output projection W_out @ g_hist runs as column-packed matmuls
(one per half-round) against the previous 8-step chunk; y is written out
in f16 and the host applies b_out.
"""

import os
import sys
import types

import numpy as np

INPUT_SIZE = 64
HIDDEN = 256
OUT = 32
NUM_TASKS = 8
ALPHA = 0.1
DECAY = 1.0 - ALPHA

B = 256
T = 2000
N_CORES = 8
N_CHAIN = 2  # staggered chains per core
SEG = 128  # real steps per chain
WARM = 48  # warmup steps per chain
STEPS = SEG + WARM  # 176
CH = 8  # steps per output chunk (one psum y bank)
NCH = SEG // CH  # 16 output chunks per chain
DMA_STEPS = 16  # steps per x DMA chunk (176 = 11*16)
EPOCH = 64  # psum rescale period (bounds the 0.9^-s scaling for fp16)
D_AUG = INPUT_SIZE + NUM_TASKS + 1  # 73 (ones row carries the bias)
YG = 4  # chunks per y DMA


def _install_ntff_hook():
    """Recreate the missing antenv.axon_hooks so trace=True can profile."""
    if "antenv.axon_hooks" in sys.modules:
        return
    mod = types.ModuleType("antenv.axon_hooks")
    mod._hook = None
    mod.set_axon_ntff_profile_hook = lambda h: setattr(mod, "_hook", h)
    mod.get_axon_ntff_profile_hook = lambda: mod._hook
    sys.modules["antenv.axon_hooks"] = mod
    try:
        from trn_agent_boot.trn_boot import _ntff_profile_via_ctypes

        mod.set_axon_ntff_profile_hook(
            _ntff_profile_via_ctypes("/opt/axon/libaxon_pjrt.so")
        )
    except Exception:
        pass


_install_ntff_hook()

import concourse.bacc as bacc
import concourse.tile as tile
import concourse.mybir as mybir
from concourse.bass_utils import run_bass_kernel_spmd

F32 = mybir.dt.float32
F16 = mybir.dt.float16

LAST_RESULT = None  # test.py reads exec_time_ns from here

_PROGRAM = None


def build_program():
    from contextlib import ExitStack

    nc = bacc.Bacc("TRN2", target_bir_lowering=False, debug=False)

    xt_d = nc.dram_tensor(
        "xt", [D_AUG, STEPS * N_CHAIN * B], F16, kind="ExternalInput"
    )
    wi_d = nc.dram_tensor("wi", [D_AUG, 2 * 128], F16, kind="ExternalInput")
    wh_d = nc.dram_tensor("wh", [128, 2 * 2 * 128], F16, kind="ExternalInput")
    whe_d = nc.dram_tensor("whe", [128, 2 * 2 * 128], F16, kind="ExternalInput")
    wo_d = nc.dram_tensor("wo", [128, 2 * OUT], F16, kind="ExternalInput")
    y_d = nc.dram_tensor(
        "y", [128, NCH * N_CHAIN * 2 * B], F16, kind="ExternalOutput"
    )

    with tile.TileContext(nc) as tc:
        ctx = ExitStack()
        with ctx:
            const = ctx.enter_context(tc.tile_pool(name="const", bufs=1))
            xpool = ctx.enter_context(tc.tile_pool(name="xin", bufs=2))
            ppool = ctx.enter_context(tc.tile_pool(name="P", bufs=1, space="PSUM"))
            ypp = ctx.enter_context(tc.tile_pool(name="ypsum", bufs=2, space="PSUM"))
            rpool = ctx.enter_context(tc.tile_pool(name="r", bufs=2))
            gpool = ctx.enter_context(tc.tile_pool(name="ghist", bufs=2))
            ysb = ctx.enter_context(tc.tile_pool(name="ysb", bufs=2))

            wi = const.tile([D_AUG, 2, 128], F16)
            nc.sync.dma_start(wi[:], wi_d.ap().rearrange("p (a m) -> p a m", a=2))
            wh = const.tile([128, 2, 2, 128], F16)
            nc.sync.dma_start(
                wh[:], wh_d.ap().rearrange("p (a b m) -> p a b m", a=2, b=2)
            )
            whe = const.tile([128, 2, 2, 128], F16)
            nc.sync.dma_start(
                whe[:], whe_d.ap().rearrange("p (a b m) -> p a b m", a=2, b=2)
            )
            wo = const.tile([128, 2, OUT], F16)
            nc.sync.dma_start(wo[:], wo_d.ap().rearrange("p (a m) -> p a m", a=2))

            # one accumulator bank per chain: [jb, b] halves side by side
            P = [
                ppool.tile([128, 2, B], F32, name=f"P{c}", tag=f"P{c}")
                for c in range(N_CHAIN)
            ]

            xt_r = xt_d.ap().rearrange(
                "p (c n) -> p c n", n=DMA_STEPS * N_CHAIN * B
            )

            r_prev = [None] * N_CHAIN
            g_hist = [None] * N_CHAIN
            g_prev = [None] * N_CHAIN
            yp_cur = [None] * N_CHAIN
            y_sbuf = [None] * N_CHAIN
            x_sbuf = None

            for s in range(STEPS):
                dc, ds = divmod(s, DMA_STEPS)  # x-DMA chunk index / step within
                if ds == 0:
                    x_sbuf = xpool.tile(
                        [D_AUG, DMA_STEPS, N_CHAIN, B], F16, tag="x"
                    )
                    nc.sync.dma_start(
                        x_sbuf.rearrange("p a c b -> p (a c b)"), xt_r[:, dc, :]
                    )

                boundary = s > 0 and s % EPOCH == 0
                whx = whe if boundary else wh

                for c in range(N_CHAIN):
                    xs = x_sbuf[:, ds, c, :]

                    if boundary:
                        resc = float(DECAY**EPOCH)
                        nc.vector.tensor_scalar_mul(P[c][:], P[c][:], resc)

                    # ---- matmul burst for chain c (writes its PSUM bank) ----
                    for jb in range(2):
                        nc.tensor.matmul(
                            P[c][:, jb, :],
                            wi[:, jb, :],
                            xs,
                            start=(s == 0),
                            stop=False,
                            skip_group_check=True,
                        )
                    if s > 0:
                        for kb in range(2):
                            for jb in range(2):
                                nc.tensor.matmul(
                                    P[c][:, jb, :],
                                    whx[:, kb, jb, :],
                                    r_prev[c][:, kb, :],
                                    start=False,
                                    stop=False,
                                    skip_group_check=True,
                                )

                    # ---- y GEMM: one matmul per half-round against the
                    # previous chunk's g_hist (fills the PE gap) ----
                    if s >= WARM + CH:
                        m = (s - WARM) % CH
                        cs = (s - WARM) // CH - 1  # chunk being projected
                        kb, t4 = divmod(m, 4)
                        if m == 0:
                            yp_cur[c] = ypp.tile(
                                [128, 2, B], F32, name=f"yp{c}", tag=f"yp{c}"
                            )
                        nc.tensor.matmul(
                            yp_cur[c][32 * t4 : 32 * (t4 + 1), :, :],
                            wo[:, kb, :],
                            g_prev[c][:, kb, 2 * t4 : 2 * t4 + 2, :],
                            start=(kb == 0),
                            stop=(kb == 1),
                            skip_group_check=True,
                            tile_position=(0, 32 * t4),
                        )

                    # ---- read P[c]: merged relu (ACT) + g_hist copy (DVE) ----
                    r_new = rpool.tile([128, 2, B], F16, name=f"r{c}", tag=f"r{c}")
                    nc.scalar.activation(
                        r_new[:], P[c][:], mybir.ActivationFunctionType.Relu
                    )
                    if s >= WARM:
                        csl = (s - WARM) % CH
                        if csl == 0:
                            g_hist[c] = gpool.tile(
                                [128, 2, CH, B], F16, name=f"g{c}", tag=f"g{c}"
                            )
                        hsc = float(DECAY ** (s % EPOCH + 1))
                        nc.vector.tensor_scalar_mul(
                            g_hist[c][:, :, csl, :], P[c][:], hsc
                        )
                    r_prev[c] = r_new

                    # ---- finished chunk bookkeeping + y copy/DMA ----
                    if s >= WARM + CH and (s - WARM) % CH == CH - 1:
                        cs = (s - WARM) // CH - 1
                        if cs % YG == 0:
                            y_sbuf[c] = ysb.tile(
                                [128, YG, 2, B],
                                F16,
                                name=f"ys{c}",
                                tag=f"ys{c}",
                            )
                        ysl = y_sbuf[c][:, cs % YG, :, :]
                        if cs % 2 == 0:
                            nc.scalar.activation(
                                ysl, yp_cur[c][:], mybir.ActivationFunctionType.Copy
                            )
                        else:
                            nc.vector.tensor_copy(ysl, yp_cur[c][:])
                        if cs % YG == YG - 1:
                            g0 = ((cs // YG) * YG * N_CHAIN + c * YG) * 2 * B
                            nc.sync.dma_start(
                                y_d.ap()[:, g0 : g0 + YG * 2 * B],
                                y_sbuf[c].rearrange("p a s b -> p (a s b)"),
                            )
                    if s >= WARM and (s - WARM) % CH == CH - 1:
                        g_prev[c] = g_hist[c]

            # ---- tail: project the final chunk of each chain ----
            for c in range(N_CHAIN):
                yp = ypp.tile([128, 2, B], F32, name=f"yp{c}", tag=f"yp{c}")
                for kb in range(2):
                    for t4 in range(4):
                        nc.tensor.matmul(
                            yp[32 * t4 : 32 * (t4 + 1), :, :],
                            wo[:, kb, :],
                            g_prev[c][:, kb, 2 * t4 : 2 * t4 + 2, :],
                            start=(kb == 0),
                            stop=(kb == 1),
                            skip_group_check=True,
                            tile_position=(0, 32 * t4),
                        )
                cs = NCH - 1
                ysl = y_sbuf[c][:, cs % YG, :, :]
                nc.scalar.activation(
                    ysl, yp[:], mybir.ActivationFunctionType.Copy
                )
                g0 = ((cs // YG) * YG * N_CHAIN + c * YG) * 2 * B
                nc.sync.dma_start(
                    y_d.ap()[:, g0 : g0 + YG * 2 * B],
                    y_sbuf[c].rearrange("p a s b -> p (a s b)"),
                )
    nc.finalize()
    return nc


def _get_program():
    global _PROGRAM
    if _PROGRAM is None:
        _PROGRAM = build_program()
    return _PROGRAM


def kernel(x, task_id, W_in, b_in, W_hh, b_hh, W_out, b_out):
    x = np.asarray(x, np.float32)
    task_id = np.asarray(task_id, np.float32)
    W_in = np.asarray(W_in, np.float32)
    b_in = np.asarray(b_in, np.float32)
    W_hh = np.asarray(W_hh, np.float32)
    b_hh = np.asarray(b_hh, np.float32)
    W_out = np.asarray(W_out, np.float32)
    b_out = np.asarray(b_out, np.float32)

    # ---- weights (shared across cores) ----
    # wi: lhsT [73, 256] = 0.1 * [W_in | b_in+b_hh]^T
    wi = np.zeros((D_AUG, HIDDEN), np.float32)
    wi[: INPUT_SIZE + NUM_TASKS, :] = ALPHA * W_in.T
    wi[INPUT_SIZE + NUM_TASKS, :] = ALPHA * (b_in + b_hh)
    # wh: lhsT [k, (kb, jb, j)] = (0.1/0.9) * W_hh[jb*128+j, kb*128+k]
    whs = (ALPHA / DECAY) * W_hh  # [j_out, k_in]
    wh = np.empty((128, 2, 2, 128), np.float32)
    for kb in range(2):
        for jb in range(2):
            wh[:, kb, jb, :] = whs[
                jb * 128 : (jb + 1) * 128, kb * 128 : (kb + 1) * 128
            ].T
    wh_in = np.ascontiguousarray(wh.reshape(128, 512)).astype(np.float16)
    # at epoch-boundary steps the relu rhs was produced before the 0.9^EPOCH
    # rescale of P, so those steps use weights pre-scaled by 0.9^EPOCH
    whe_in = np.ascontiguousarray(wh.reshape(128, 512) * (DECAY**EPOCH)).astype(
        np.float16
    )
    # wo: lhsT [k, (kb, o)] = W_out[o, kb*128+k]
    wo = np.empty((128, 2, OUT), np.float32)
    for kb in range(2):
        wo[:, kb, :] = W_out[:, kb * 128 : (kb + 1) * 128].T
    wo_in = np.ascontiguousarray(wo.reshape(128, 2 * OUT)).astype(np.float16)

    # ---- per-core scaled input blocks ----
    # combined_aug[d, t, b]: [73, T, B]
    comb = np.concatenate(
        [x, np.broadcast_to(task_id[:, None, :], (B, T, NUM_TASKS))], axis=2
    )  # [B, T, 72]
    comb_t = comb.transpose(2, 1, 0)  # [72, T, B]
    # per-step scale 0.9^-(s+1) with s local to each chain
    sc = (
        DECAY ** -(np.arange(STEPS, dtype=np.float64) % EPOCH + 1)
    ).astype(np.float32)

    in_maps = []
    for core in range(N_CORES):
        xt = np.zeros((D_AUG, STEPS, N_CHAIN, B), np.float32)
        for c in range(N_CHAIN):
            seg0 = (N_CHAIN * core + c) * SEG  # global start of this segment
            t0 = seg0 - WARM
            lo = max(t0, 0)
            hi = min(seg0 + SEG, T)
            if hi > lo:
                ls, le = lo - t0, hi - t0
                xt[: INPUT_SIZE + NUM_TASKS, ls:le, c, :] = comb_t[:, lo:hi, :]
                xt[INPUT_SIZE + NUM_TASKS, ls:le, c, :] = 1.0
        xt *= sc[None, :, None, None]
        in_maps.append(
            {
                "xt": np.ascontiguousarray(
                    xt.reshape(D_AUG, STEPS * N_CHAIN * B)
                ).astype(np.float16),
                "wi": np.ascontiguousarray(wi).astype(np.float16),
                "wh": wh_in,
                "whe": whe_in,
                "wo": wo_in,
            }
        )

    nc = _get_program()
    global LAST_RESULT
    trace = bool(int(os.environ.get("KERNEL_TRACE", "0")))
    LAST_RESULT = run_bass_kernel_spmd(
        nc, in_maps, core_ids=list(range(N_CORES)), trace=trace
    )

    # ---- gather: y[(t4,o), (chunkgrp, chunk%YG, chain, s2, b)] -> [B,T,OUT] ----
    out = np.empty((B, T, OUT), np.float32)
    for core in range(N_CORES):
        y = LAST_RESULT.results[core]["y"].astype(np.float32)
        # [(t4, o), (cg, chain, cl, s2, b)]
        y = y.reshape(4, OUT, NCH // YG, N_CHAIN, YG, 2, B)
        for c in range(N_CHAIN):
            yc = y[:, :, :, c, :, :, :]  # [t4, o, cg, cl, s2, b]
            # seg step = (cg*YG + cl)*8 + t4*2 + s2
            yc = yc.transpose(2, 3, 0, 4, 1, 5).reshape(SEG, OUT, B)
            yc = yc + b_out[None, :, None]
            seg0 = (N_CHAIN * core + c) * SEG
            n = min(SEG, T - seg0)
            if n > 0:
                out[:, seg0 : seg0 + n, :] = yc[:n].transpose(2, 0, 1)
    return out
